# revision 1
# baseline (speedup 1.0000x reference)
"""MixedScoreMultiHeadAttention Trainium2 kernel (PE-centric pipeline).

Data-parallel over batch: 32 batches -> 8 cores x 4 batches.

Per (b):
  dot_h = q_h k_h^T  (per head, PE)  -> flattened r-major into rhs rows
  layer1: T[(h,m), pts] = a[h,m]*dot_h[pts] + c[h,m]*Y[pts]   (PE matmul,
          constant stationary [17,128]; bias b[h,m] folded into relu evac)
  R = relu(T + b)                     (ACT/DVE evacuation from PSUM, fp16)
  layer2: mixed^T[pts, h] via stationary-swapped matmul (lhsT = R data,
          rhs = block-diag w2 [128,8]) -> PSUM [c, (r-grp, h)] full-partition
  exp-evac (ACT Exp) -> w_sb [c, (r,h)] fp32
  AV: out[r, 17] = w^T-slice.T @ [v|1]  (ones col gives softmax denominator)
  normalize by reciprocal of col 16 -> out rows

mix2 bias b2 is dropped (constant shift is softmax-invariant); 1/sqrt(D) is
folded into Wq host-side.
"""
import sys

sys.path.insert(0, "/opt/trn_rl_repo")

import numpy as np
from contextlib import ExitStack

import concourse.bass as bass
import concourse.mybir as mybir
import concourse.tile as tile
from concourse import bacc
from concourse.bass_utils import run_bass_kernel_spmd
from concourse.masks import make_identity

B, R, C, E, H, D, MS = 32, 128, 128, 256, 16, 16, 16
NCORES = 8
BL = B // NCORES  # batches per core: 4
TOK = BL * R      # 512 tokens per core per side
PTS = R * C       # 16384 score points per (b)

FP32 = mybir.dt.float32
FP16 = mybir.dt.float16
AF = mybir.ActivationFunctionType
ALU = mybir.AluOpType



def build_kernel():
    nc = bacc.Bacc("TRN2", target_bir_lowering=False, debug=False,
                   num_devices=NCORES)

    x_r = nc.dram_tensor("x_r", [TOK, E], FP32, kind="ExternalInput").ap()
    x_c = nc.dram_tensor("x_c", [TOK, E], FP32, kind="ExternalInput").ap()
    cost = nc.dram_tensor("cost", [BL, R, C], FP32, kind="ExternalInput").ap()
    # Wq pre-scaled by 1/sqrt(D) host-side; head-padding to 32-col slots
    # (for 32-aligned projection PSUM rows) happens on-chip.
    wq_d = nc.dram_tensor("Wq", [E, E], FP32, kind="ExternalInput").ap()
    wk_d = nc.dram_tensor("Wk", [E, E], FP32, kind="ExternalInput").ap()
    wv_d = nc.dram_tensor("Wv", [E, E], FP32, kind="ExternalInput").ap()
    # layer1 stationary [17, 256]: col (half*128 + (h%8)*16 + m):
    #   row h' = a[h,m] iff h'==h; row 16 = c[h,m]
    w1_d = nc.dram_tensor("W1L", [17, 2 * 128], FP32,
                          kind="ExternalInput").ap()
    # layer2 moving [128, 16]: col (half*8 + j): row hm = w2[half*8+j, m]
    # iff hm == ((j)*16+m) else 0
    w2_d = nc.dram_tensor("W2L", [128, 16], FP32, kind="ExternalInput").ap()
    # relu bias per (h,m) row: bcol2[hm, half] = b1[half*8 + hm//16, hm%16]
    bc_d = nc.dram_tensor("bcol2", [128, 2], FP32, kind="ExternalInput").ap()
    out_d = nc.dram_tensor("out", [BL, R, H * D], FP32,
                           kind="ExternalOutput").ap()

    with tile.TileContext(nc) as tc, ExitStack() as ctx:
        const_p = ctx.enter_context(tc.tile_pool(name="const", bufs=1))
        inx_p = ctx.enter_context(tc.tile_pool(name="inx", bufs=2))
        w_p = ctx.enter_context(tc.tile_pool(name="wts", bufs=1))
        xt_p = ctx.enter_context(tc.tile_pool(name="xt", bufs=1))
        qkv_p = ctx.enter_context(tc.tile_pool(name="qkv", bufs=1))
        x4_p = ctx.enter_context(tc.tile_pool(name="x4", bufs=1))
        rhs_p = ctx.enter_context(tc.tile_pool(name="rhs", bufs=2))
        rr_p = ctx.enter_context(tc.tile_pool(name="rr", bufs=6))
        wsb_p = ctx.enter_context(tc.tile_pool(name="wsb", bufs=2))
        fout_p = ctx.enter_context(tc.tile_pool(name="fout", bufs=1))
        small_p = ctx.enter_context(tc.tile_pool(name="small", bufs=4))
        ps_tr = ctx.enter_context(
            tc.tile_pool(name="pstr", bufs=1, space="PSUM"))
        ps_big = ctx.enter_context(
            tc.tile_pool(name="psb", bufs=4, space="PSUM"))
        ps_l2 = ctx.enter_context(
            tc.tile_pool(name="psl2", bufs=2, space="PSUM"))
        ps_av = ctx.enter_context(
            tc.tile_pool(name="psa", bufs=1, space="PSUM"))

        ident = const_p.tile([128, 128], FP32)
        make_identity(nc, ident[:])

        # ---- small weight/const loads
        w1f = inx_p.tile([17, 2 * 128], FP32, tag="w1f")
        nc.sync.dma_start(w1f[:], w1_d[:])
        w1l = const_p.tile([17, 2 * 128], FP16)
        nc.vector.tensor_copy(w1l[:], w1f[:])

        w2f = inx_p.tile([128, 16], FP32, tag="w2f")
        nc.sync.dma_start(w2f[:], w2_d[:])
        w2l = const_p.tile([128, 16], FP16)
        nc.vector.tensor_copy(w2l[:], w2f[:])

        bcol2 = const_p.tile([128, 2], FP32)
        nc.sync.dma_start(bcol2[:], bc_d[:])

        # ---- QKV weights fp16 (q/k padded on-chip: head h -> 32-col slot)
        wt16 = {}
        for name, dram in (("q", wq_d), ("k", wk_d), ("v", wv_d)):
            halves = []
            for eh in range(2):
                w32 = inx_p.tile([128, E], FP32, tag="wload")
                nc.sync.dma_start(w32[:], dram[eh * 128:(eh + 1) * 128, :])
                ncols = E if name == "v" else 2 * E
                w16 = w_p.tile([128, ncols], FP16, tag=f"w16{name}{eh}",
                               name=f"w16{name}{eh}")
                if name == "v":
                    nc.vector.tensor_copy(w16[:], w32[:])
                else:
                    nc.gpsimd.memset(w16[:], 0.0)
                    w16v = w16[:].rearrange("p (h x) -> p h x", h=H)
                    w32v = w32[:].rearrange("p (h x) -> p h x", h=H)
                    nc.vector.tensor_copy(w16v[:, :, 0:D], w32v[:])
                halves.append(w16)
            wt16[name] = halves

        # ---- x load + PE transpose -> xT fp16 [2 e-halves][128, TOK]
        xT = {}
        for name, dram in (("r", x_r), ("c", x_c)):
            xt0 = xt_p.tile([128, TOK], FP16, tag=f"xT{name}0")
            xt1 = xt_p.tile([128, TOK], FP16, tag=f"xT{name}1")
            xT[name] = [xt0, xt1]
            for t in range(BL):
                x32 = inx_p.tile([128, E], FP32, tag="xload")
                nc.sync.dma_start(x32[:], dram[t * 128:(t + 1) * 128, :])
                for eh in range(2):
                    pst = ps_tr.tile([128, 128], FP32, tag="pstr")
                    nc.tensor.transpose(
                        pst[:], x32[:, eh * 128:(eh + 1) * 128], ident[:])
                    nc.vector.tensor_copy(
                        xT[name][eh][:, t * 128:(t + 1) * 128], pst[:])

        # ---- cost -> fp16 [r, c] tiles (r-major flatten later)
        y16 = []
        for b in range(BL):
            c32 = inx_p.tile([128, C], FP32, tag="cload")
            nc.sync.dma_start(c32[:], cost[b])
            y1 = const_p.tile([128, C], FP16, name=f"y16_{b}", tag=f"y16_{b}")
            nc.vector.tensor_copy(y1[:], c32[:])
            y16.append(y1)

        # ---- projections: qT/kT per-head tiles [16, TOK] fp16
        qT, kT = [], []
        for proj, dst in (("q", qT), ("k", kT)):
            for mh in range(4):  # head-quad tiles (4 heads x 32 rows)
                ps = ps_big.tile([128, TOK], FP32, tag="psbig")
                for eh in range(2):
                    nc.tensor.matmul(
                        ps[:],
                        wt16[proj][eh][:, mh * 128:(mh + 1) * 128],
                        xT["r" if proj == "q" else "c"][eh][:],
                        start=(eh == 0), stop=(eh == 1))
                # pack 3 head evacs in one 96-row op (PE matmul operands may
                # sit at base partition 0/32/64; 96 is invalid -> separate)
                quad = qkv_p.tile([96, TOK], FP16, tag=f"{proj}Q{mh}",
                                  name=f"{proj}Q{mh}")
                last = qkv_p.tile([16, TOK], FP16, tag=f"{proj}L{mh}",
                                  name=f"{proj}L{mh}")
                if mh % 2 == 0:
                    nc.scalar.copy(quad[:], ps[0:96, :])
                    nc.vector.tensor_copy(last[:], ps[96:112, :])
                else:
                    nc.vector.tensor_copy(quad[:], ps[0:96, :])
                    nc.scalar.copy(last[:], ps[96:112, :])
                for hh in range(4):
                    dst.append(quad[hh * 32:hh * 32 + 16, :] if hh < 3
                               else last[:])

        # ---- v natural [c, hd] fp32 interleaved with ones col -> vhat
        vhat = []
        for b in range(BL):
            vh = qkv_p.tile([128, 17 * H], FP32, tag=f"vhat{b}",
                            name=f"vhat{b}")
            vh3 = vh[:].rearrange("p (h x) -> p h x", h=H)
            nc.gpsimd.memset(vh3[:, :, 16:17], 1.0)
            ps = ps_big.tile([128, E], FP32, tag="psbig")
            for eh in range(2):
                nc.tensor.matmul(
                    ps[:], xT["c"][eh][:, b * 128:(b + 1) * 128],
                    wt16["v"][eh][:], start=(eh == 0), stop=(eh == 1))
            nc.scalar.copy(
                vh3[:, :, 0:16], ps[:].rearrange("p (h x) -> p h x", h=H))
            vhat.append(vh)

        # ---- dots: X4[h] fp16 [r, (b, c)]
        x4s = []
        for h in range(H):
            x4 = x4_p.tile([128, BL * C], FP16, tag=f"x4_{h}",
                           name=f"x4_{h}")
            psd = ps_big.tile([128, BL * C], FP32, tag="psbig")
            for b in range(BL):
                nc.tensor.matmul(
                    psd[:, b * 128:(b + 1) * 128],
                    qT[h][:, b * 128:(b + 1) * 128],
                    kT[h][:, b * 128:(b + 1) * 128])
            if h % 2 == 0:
                nc.scalar.copy(x4[:], psd[:])
            else:
                nc.vector.tensor_copy(x4[:], psd[:])
            x4s.append(x4)

        # ---- per (b): layer1+relu, layer2, exp, AV
        fouts = [fout_p.tile([128, H * D], FP32, tag=f"fo{b}", name=f"fo{b}")
                 for b in range(BL)]
        for b in range(BL):
            rhs = rhs_p.tile([17, PTS], FP16, tag="rhs")
            for h in range(H):
                nc.sync.dma_start(rhs[h:h + 1, :],
                                  x4s[h][:, b * 128:(b + 1) * 128])
            nc.sync.dma_start(rhs[16:17, :], y16[b][:])

            for half in range(2):
                wsb = wsb_p.tile([128, 8 * C], FP32)
                for grp in range(2):  # 64 r's per group
                    ps2 = ps_l2.tile([128, 512], FP32)
                    for cki in range(16):  # layer1 chunks of 512 pts
                        ck = grp * 16 + cki
                        rr = rr_p.tile([128, 512], FP16, tag="rr")
                        ps1 = ps_big.tile([128, 512], FP32, tag="psbig")
                        nc.tensor.matmul(
                            ps1[:], w1l[:, half * 128:(half + 1) * 128],
                            rhs[:, ck * 512:(ck + 1) * 512])
                        if ck % 2 == 0:
                            nc.scalar.activation(
                                rr[:], ps1[:], AF.Relu,
                                bias=bcol2[:, half:half + 1])
                        else:
                            nc.vector.tensor_scalar(
                                rr[:], ps1[:], bcol2[:, half:half + 1],
                                0.0, ALU.add, ALU.max)
                        for s in range(4):  # layer2 per 128-pt subchunk
                            rloc = cki * 4 + s
                            nc.tensor.matmul(
                                ps2[:, rloc * 8:rloc * 8 + 8],
                                rr[:, s * 128:(s + 1) * 128],
                                w2l[:, half * 8:(half + 1) * 8])
                    nc.scalar.activation(
                        wsb[:, grp * 512:(grp + 1) * 512], ps2[:], AF.Exp)

                # AV + normalize for the 8 heads of this half
                psa = ps_av.tile([128, 17 * 8], FP32)
                wsb4 = wsb[:].rearrange("p (g s h) -> p g s h", g=2, s=64)
                for hl in range(8):
                    h = half * 8 + hl
                    nc.tensor.matmul(
                        psa[:, hl * 17:(hl + 1) * 17],
                        wsb4[:, :, :, hl],
                        vhat[b][:, h * 17:(h + 1) * 17])
                rec = small_p.tile([128, 8], FP32, tag="rec")
                psa3 = psa[:].rearrange("p (x y) -> p x y", x=8)
                nc.vector.reciprocal(rec[:], psa3[:, :, 16])
                for hl in range(8):
                    h = half * 8 + hl
                    nc.vector.tensor_scalar(
                        fouts[b][:, h * D:(h + 1) * D], psa3[:, hl, 0:16],
                        rec[:, hl:hl + 1], None, ALU.mult)

        for b in range(BL):
            nc.sync.dma_start(out_d[b], fouts[b][:])

    nc.compile()
    return nc


_cache = {}


def kernel(**inputs):
    row_emb = np.asarray(inputs["row_emb"], dtype=np.float32)
    col_emb = np.asarray(inputs["col_emb"], dtype=np.float32)
    cost_mat = np.asarray(inputs["cost_mat"], dtype=np.float32)
    Wq = np.asarray(inputs["Wq"], dtype=np.float32)
    Wk = np.asarray(inputs["Wk"], dtype=np.float32)
    Wv = np.asarray(inputs["Wv"], dtype=np.float32)
    m1w = np.asarray(inputs["mix1_weight"], dtype=np.float32)
    m1b = np.asarray(inputs["mix1_bias"], dtype=np.float32)
    m2w = np.asarray(inputs["mix2_weight"], dtype=np.float32)

    a1 = m1w[:, 0, :]
    c1 = m1w[:, 1, :]
    w2 = m2w[:, :, 0]

    if "nc" not in _cache:
        _cache["nc"] = build_kernel()
    nc = _cache["nc"]

    wq_s = Wq * (1.0 / np.sqrt(D))
    wk_p = Wk

    w1l = np.zeros((17, 256), dtype=np.float32)
    w2l = np.zeros((128, 16), dtype=np.float32)
    bcol2 = np.zeros((128, 2), dtype=np.float32)
    for h in range(H):
        half, hl = h // 8, h % 8
        for m in range(MS):
            col = half * 128 + hl * 16 + m
            w1l[h, col] = a1[h, m]
            w1l[16, col] = c1[h, m]
            w2l[hl * 16 + m, half * 8 + hl] = w2[h, m]
            bcol2[hl * 16 + m, half] = m1b[h, m]

    in_maps = []
    for i in range(NCORES):
        sl = slice(i * BL, (i + 1) * BL)
        in_maps.append({
            "x_r": row_emb[sl].reshape(TOK, E),
            "x_c": col_emb[sl].reshape(TOK, E),
            "cost": cost_mat[sl],
            "Wq": wq_s, "Wk": wk_p, "Wv": Wv,
            "W1L": w1l, "W2L": w2l, "bcol2": bcol2,
        })
    res = run_bass_kernel_spmd(nc, in_maps, list(range(NCORES)))
    out = np.concatenate([res.results[i]["out"] for i in range(NCORES)],
                         axis=0)
    return out.astype(np.float32)

